# revision 1
# baseline (speedup 1.0000x reference)
"""Overlapping-windows kernel (tf.nn.conv1d with identity filter) for TRN2.

Full input x: [64, 2000, 26] f32. Full output: [64, 2000, 494] f32 where
out[b, t, w*26 + c] = x_pad[b, t + w, c]  (x zero-padded by 9 frames each side).

Sharding: pure data parallel over batch — 8 examples per NeuronCore, 8 cores.

Per-core kernel (x_shard [8, 2000, 26] -> y_shard [8, 2000, 494]):
  Key observation: out[b, t, :] = x[b, t-9 : t+10, :].flatten() — each output
  row is a CONTIGUOUS 494-float slice of x[b] (row pitch 26 floats).

  Stage 1 (load): partition p = e*16 + k holds input rows
  [k*125-9, k*125+134) of example e (125 output rows + 9-row halos),
  flattened to 3718 floats; out-of-range halos zeroed by memset. Loads are
  per-example DMAs split across both HWDGE rings (sync + scalar).
  (SBUF-side DMA access patterns must keep ap[0] as the partition dim with
  step == row pitch; leading dims that hop multiple partitions silently
  corrupt addressing on HW.)

  Stage 2 (expand): DVE expands the 19 overlapping windows per output row
  into contiguous per-partition runs — one fused 4-dim-AP tensor_copy per
  chunk, 6 uneven chunks (small first chunk so the store pipe starts early)
  rotating through 3 buffers.

  Stage 3 (store): per chunk, one DMA writes [128 partitions x contiguous
  run] to y — big descriptors run at HBM line rate (a direct
  overlapping-window DMA with 1976B descriptors is ~2.4x slower per byte).
  Chunks alternate between the two HWDGE rings. WAR reuse of each buffer is
  gated by a per-buffer semaphore (a shared semaphore cannot distinguish
  WHICH of two outstanding DMAs completed).

  HBM traffic per core: 1.7 MB read + 31.6 MB write. Measured ~110-124 us
  (vs ~88 us write roofline; ~168 us for the no-expansion direct DMA).
"""

from contextlib import ExitStack

import numpy as np

import concourse.bass as bass
import concourse.mybir as mybir
from concourse.bass_utils import run_bass_kernel_spmd

# Problem constants (hardcoded per contract)
B_FULL = 64
T = 2000
C = 26
NCTX = 9
W = 2 * NCTX + 1          # 19
WC = W * C                # 494
N_CORES = 8
BL = B_FULL // N_CORES    # 8 examples per core
K = 16                    # row-chunks per example -> BL*K = 128 partitions
R = T // K                # 125 output rows per partition
FL = (R + 2 * NCTX) * C   # 3718 floats per partition (125+18 rows * 26)
HALO = NCTX * C           # 234 floats of halo on each side
XROW = T * C              # 52000 floats per example in x
YROW = T * WC             # 988000 floats per example in y
F32 = mybir.dt.float32

CHUNKS = (5, 24, 24, 24, 24, 24)  # output rows per expansion chunk
NBUF = 3                          # expansion ping-pong buffers


def _build():
    nchunk = len(CHUNKS)
    outw = max(CHUNKS) * WC
    starts = [sum(CHUNKS[:i]) for i in range(nchunk)]
    nc = bass.Bass()
    x = nc.dram_tensor("x", [BL, T, C], F32, kind="ExternalInput")
    y = nc.dram_tensor("y", [BL, T, WC], F32, kind="ExternalOutput")

    with ExitStack() as ctx:
        tile = ctx.enter_context(nc.sbuf_tensor("tile", [128, FL], F32))
        obufs = [ctx.enter_context(
                     nc.sbuf_tensor(f"obuf{i}", [128, outw], F32))
                 for i in range(NBUF)]
        vsem = ctx.enter_context(nc.semaphore("vsem"))
        lsemA = ctx.enter_context(nc.semaphore("lsemA"))
        lsemB = ctx.enter_context(nc.semaphore("lsemB"))
        esem = ctx.enter_context(nc.semaphore("esem"))
        osems = [ctx.enter_context(nc.semaphore(f"osem{i}"))
                 for i in range(NBUF)]
        block = ctx.enter_context(nc.Block())
        th = tile[:].tensor
        xt = x[:].tensor

        def half_loads(eng, es, lsem):
            for e in es:
                # interior chunks k=1..14: 14 contiguous partitions
                src = bass.AP(tensor=xt, offset=e * XROW + R * C - HALO,
                              ap=[[R * C, K - 2], [1, FL]])
                dst = bass.AP(tensor=th, offset=(e * K + 1) * FL,
                              ap=[[FL, K - 2], [1, FL]])
                eng.dma_start(out=dst, in_=src).then_inc(lsem, 16)
                # k=0: rows [0,134) -> partition e*16, cols [234, 3718)
                src0 = bass.AP(tensor=xt, offset=e * XROW,
                               ap=[[1, FL - HALO]])
                dst0 = bass.AP(tensor=th, offset=(e * K) * FL + HALO,
                               ap=[[FL, 1], [1, FL - HALO]])
                eng.dma_start(out=dst0, in_=src0).then_inc(lsem, 16)
                # k=15: rows [1866,2000) -> partition e*16+15, cols [0,3484)
                src15 = bass.AP(tensor=xt,
                                offset=e * XROW + (K - 1) * R * C - HALO,
                                ap=[[1, FL - HALO]])
                dst15 = bass.AP(tensor=th, offset=(e * K + K - 1) * FL,
                                ap=[[FL, 1], [1, FL - HALO]])
                eng.dma_start(out=dst15, in_=src15).then_inc(lsem, 16)

        def out_dma(eng, c):
            ob = obufs[c % NBUF][:].tensor
            cn = CHUNKS[c]
            src = bass.AP(tensor=ob, offset=0, ap=[[outw, 128], [1, cn * WC]])
            dst = bass.AP(tensor=y[:].tensor, offset=starts[c] * WC,
                          ap=[[R * WC, 128], [1, cn * WC]])
            eng.dma_start(out=dst, in_=src).then_inc(osems[c % NBUF], 16)

        @block.vector
        def _(vector):
            # Zero halo columns on all partitions (engines need aligned
            # start partitions); loads then overwrite non-halo spans.
            vector.memset(tile[:, 0:HALO], 0.0).then_inc(vsem, 1)
            vector.memset(tile[:, FL - HALO:FL], 0.0).then_inc(vsem, 1)
            vector.wait_ge(lsemA, 16 * 12)
            vector.wait_ge(lsemB, 16 * 12)
            for c in range(nchunk):
                if c >= NBUF:
                    # WAR: all prior out-DMAs of this buffer completed.
                    # Sound because this wait serializes per-buffer DMAs.
                    vector.wait_ge(osems[c % NBUF], 16 * (c // NBUF))
                ob = obufs[c % NBUF][:].tensor
                cn = CHUNKS[c]
                # ob[p, t*494 + w*26 + cc] = tile[p, (start + t + w)*26 + cc]
                src = bass.AP(tensor=th, offset=starts[c] * C,
                              ap=[[FL, 128], [C, cn], [C, W], [1, C]])
                dst = bass.AP(tensor=ob, offset=0,
                              ap=[[outw, 128], [WC, cn], [C, W], [1, C]])
                vector.tensor_copy(out=dst, in_=src).then_inc(esem, 1)

        @block.sync
        def _(sync):
            sync.wait_ge(vsem, 2)
            half_loads(sync, range(0, BL, 2), lsemA)
            for c in range(0, nchunk, 2):
                sync.wait_ge(esem, c + 1)
                out_dma(sync, c)
            for b in range(NBUF):
                ntot = len([c for c in range(nchunk) if c % NBUF == b])
                sync.wait_ge(osems[b], 16 * ntot)

        @block.scalar
        def _(scalar):
            scalar.wait_ge(vsem, 2)
            half_loads(scalar, range(1, BL, 2), lsemB)
            for c in range(1, nchunk, 2):
                scalar.wait_ge(esem, c + 1)
                out_dma(scalar, c)

    return nc


_NC = None


def _get_nc():
    global _NC
    if _NC is None:
        _NC = _build()
    return _NC


def run(x: np.ndarray, trace: bool = False):
    """Run the kernel on all 8 cores; returns (y_full, BassKernelResults)."""
    x = np.ascontiguousarray(x, dtype=np.float32)
    assert x.shape == (B_FULL, T, C), x.shape
    nc = _get_nc()
    in_maps = [
        {"x": x[i * BL:(i + 1) * BL]} for i in range(N_CORES)
    ]
    res = run_bass_kernel_spmd(
        nc, in_maps, core_ids=list(range(N_CORES)), trace=trace
    )
    y = np.concatenate([res.results[i]["y"] for i in range(N_CORES)], axis=0)
    return y, res


def kernel(x: np.ndarray) -> np.ndarray:
    y, _ = run(x)
    return y

